# revision 22
# baseline (speedup 1.0000x reference)
"""Trainium2 Bass kernel for nn_MultiHeadAttention_42752104464925.

Multi-head attention (VITS-style) with windowed relative position embeddings
(window=4, heads_share=True).

Math notes
----------
With L=1024, WIN=4, the relative-key logits term rel_to_abs(q_scaled @ rel_k^T)
is a 9-diagonal band:   scores[t,s] += q_scaled[t] . emb_k[s-t+4]   (|s-t|<=4)
and the relative-value term is:
  out[t] += sum_j p[t, t+j-4] * emb_v[j]   (0 <= t+j-4 < L)

Sharding: 8 cores = 4 batches x 2 head-groups (6 heads each). Each core
computes QKV for its 384 channels, attention for its 6 heads, and a partial
output projection Wo[:, slice] @ merged. Host sums the two partials per batch.

Device layout per head: scores are computed TRANSPOSED (S^T[s,t], s on
partitions) so that A@V needs no transpose of the attention weights; the
softmax denominator L[t] (a partition-dim sum) comes for free from a ones
column appended to the V^T stationary operand (M=66).

Schedule notes (perf):
* The whole datapath is bf16 (fp32 PSUM accumulation, fp32 output): halves
  the HBM load traffic (which paces phase A) and the band staging DMAs, and
  frees enough SBUF to double-buffer the exp(scores) tile so consecutive
  head pairs overlap.
* All large DRAM inputs are host-pre-tiled to [128, ...] partition-major so
  each load is 128 long descriptor runs; small tensors load on the sync
  queue first (the gpsimd queue starts ~20us late due to the library load).
* Identity warmup matmuls run during the input DMAs so the PE HAM clock
  gate is at 8/8 when the projections start.
* band_k is applied on the PE: the per-chunk band window wt6 (staged via the
  skewed DRAM regions rk*) is accumulated into the scores PSUM group as an
  identity-stationary matmul, so the scores->exp chain never leaves
  TensorE->ScalarE.
* A@V matmuls for half-chunk (c,n) are emitted one half-chunk late so the
  in-order PE queue never blocks on exp(c,n); scores(c,n+1) fills the gap.
* band_v: the 9 diagonals of exp(S) are read back from the es window staging
  as a compact [128, 18] tile, transposed on TensorE, shear-aligned with 9
  SBUF->SBUF DMAs (row-permuting with stride-9 partition pairs), and folded
  into the A@V PSUM accumulation with K=18 matmuls against a
  host-precomputed embedding matrix.
"""

import math

import numpy as np
import ml_dtypes

import concourse.bacc as bacc
import concourse.bass as bass
import concourse.mybir as mybir
import concourse.tile as tile
from concourse import library_config
from concourse.bass_utils import run_bass_kernel_spmd

# Problem constants (hardcoded per harness contract).
B, C, T, H, KC, WIN = 4, 768, 1024, 12, 64, 4
HL = 6            # heads per core
CL = HL * KC      # 384 local channels
NSUB = C // 128   # 6 k-subtiles over C
LSUB = CL // 128  # 3 subtiles over CL
NCH = T // 128    # 8 s-chunks
NB = 9            # band width (2*WIN+1)
WQ = 136          # band window width per 128-chunk (128 + 2*WIN)
# band_k skewed staging: region element addr = RK_GUARD + RK_ROW*p + 6*f + h
# holds W[p, f, h] = Rt_h[t0+f, p-f+8]; rows are 816 elements used of RK_ROW.
RK_ROW = 6 * 144
RK_GUARD = 6912
RK_LEN = RK_GUARD + RK_ROW * WQ
W2_ROWS, W2_COLS = 144, 136  # padded expS window staging (8 zero rows each end)
W2_REG = W2_ROWS * W2_COLS   # one head's region; a pair shares one tensor

F32 = mybir.dt.float32
BF = mybir.dt.bfloat16
NPBF = ml_dtypes.bfloat16
AF = mybir.ActivationFunctionType
ALU = mybir.AluOpType

N_WARMUP = 200     # warmup matmuls issued during the load phase
F16 = mybir.dt.float16


def _raw(t_ap, off, dims):
    """Raw element-offset AP into (the tensor behind) an AP."""
    return bass.AP(tensor=t_ap.tensor, offset=t_ap.offset + off, ap=dims)


def _chunk_window(c):
    """Clipped t-window [t_lo, t_lo+w) for s-chunk c; q0 = offset into the
    unclipped 136-wide window starting at t0 = 128c - 4."""
    t0 = 128 * c - WIN
    t_lo = max(0, t0)
    q0 = t_lo - t0
    w = min(T, t0 + WQ) - t_lo
    return t_lo, q0, w


def _half_segments(c, n):
    """Absolute-t segments of chunk c's band window inside half n."""
    t_lo, q0, w = _chunk_window(c)
    a = max(t_lo, 512 * n)
    b = min(t_lo + w, 512 * (n + 1))
    return [(a, b)] if a < b else []


def _bandv_segments():
    """(c, a, b) absolute-t segments of each chunk's band window, split at
    PSUM bank (512) boundaries."""
    segs = []
    for c in range(NCH):
        t0 = 128 * c - WIN
        a, b = max(t0, 0), min(t0 + WQ, T)
        cuts = [a] + [x for x in (512,) if a < x < b] + [b]
        for k in range(len(cuts) - 1):
            segs.append((c, cuts[k], cuts[k + 1]))
    return segs


def build_program():
    nc = bacc.Bacc("TRN2", target_bir_lowering=False, debug=False,
                   enable_asserts=True)

    # ---- I/O ----  (big tensors host-pre-tiled to [128, ...], bf16)
    xb = nc.dram_tensor("xb", [128, NSUB * T], BF, kind="ExternalInput")
    cb = nc.dram_tensor("cb", [128, NSUB * T], BF, kind="ExternalInput")
    wqt = nc.dram_tensor("wqt", [128, NSUB * CL], BF, kind="ExternalInput")
    wkt = nc.dram_tensor("wkt", [128, NSUB * CL], BF, kind="ExternalInput")
    wvt = nc.dram_tensor("wvt", [128, NSUB * CL], BF, kind="ExternalInput")
    wot = nc.dram_tensor("wot", [128, LSUB * C], BF, kind="ExternalInput")
    bq2 = nc.dram_tensor("bq2", [128, LSUB], F32, kind="ExternalInput")
    bk2 = nc.dram_tensor("bk2", [128, LSUB], F32, kind="ExternalInput")
    bvr = nc.dram_tensor("bvr", [128, CL], F32, kind="ExternalInput")
    ekt18d = nc.dram_tensor("ekt18", [128, 2 * NB], BF, kind="ExternalInput")
    ev18d = nc.dram_tensor("ev18", [2 * NB, 128], BF, kind="ExternalInput")
    i128d = nc.dram_tensor("i128", [128, 128], BF, kind="ExternalInput")
    ones8 = nc.dram_tensor("ones8", [128, NCH, 2], BF, kind="ExternalInput")
    # zero-padded staging buffers (host supplies zeros; device writes data)
    rk = [nc.dram_tensor(f"rk{c}", [RK_LEN], BF, kind="ExternalInput")
          for c in range(NCH)]
    w2 = [nc.dram_tensor(f"w2_{i}", [2 * W2_REG], BF, kind="ExternalInput")
          for i in range((HL // 2) * NCH)]
    outp = nc.dram_tensor("outp", [C, T], F16, kind="ExternalOutput")

    with tile.TileContext(nc) as tc:
        nc.gpsimd.load_library(library_config.attn)
        with tc.tile_pool(name="persist", bufs=1) as pp:
            # persistent SBUF
            q_sb = pp.tile([128, LSUB, T], BF, tag="q_sb")
            k_sb = pp.tile([128, LSUB, T], BF, tag="k_sb")
            vt = [pp.tile([128, NCH, KC + 2], BF, tag=f"vt{h}", name=f"vt{h}")
                  for h in range(HL)]
            wo_sb = pp.tile([128, LSUB, C], BF, tag="wo_sb")
            merged = pp.tile([128, LSUB, T], BF, tag="merged")
            ekt_sb = pp.tile([128, 2 * NB], BF, tag="ekt_sb")
            ev18_sb = pp.tile([2 * NB, 128], BF, tag="ev18_sb")
            i128_sb = pp.tile([128, 128], BF, tag="i128_sb")
            bq_sb = pp.tile([128, LSUB], F32, tag="bq_sb")
            bk_sb = pp.tile([128, LSUB], F32, tag="bk_sb")
            bv_sb = pp.tile([128, CL], F32, tag="bv_sb")
            # band_k windows for all chunks/heads: [p, c, f, h]
            wt6 = pp.tile([128, NCH, WQ, HL], BF, tag="wt6")

            # ---------------- Phase A: projections ----------------
            with tc.tile_pool(name="pa", bufs=1) as pa, \
                 tc.tile_pool(name="pa_ps", bufs=1, space="PSUM") as pa_ps, \
                 tc.tile_pool(name="pa_ps2", bufs=2, space="PSUM") as pa_ps2, \
                 tc.tile_pool(name="pa_ps3", bufs=2, space="PSUM") as pa_ps3:
                x_sb = pa.tile([128, NSUB, T], BF, tag="x_sb")
                c_sb = pa.tile([128, NSUB, T], BF, tag="c_sb")
                wq_sb = pa.tile([128, NSUB, CL], BF, tag="wq_sb")
                wk_sb = pa.tile([128, NSUB, CL], BF, tag="wk_sb")
                wv_sb = pa.tile([128, NSUB, CL], BF, tag="wv_sb")
                # Rt staging, head-interleaved: [t_part, c, j, h]
                rts = pa.tile([128, NCH, NB, HL], BF, tag="rts")

                # PE warmup: a memset SBUF tile needs no DMA, so the PE
                # starts within ~1.5us and the HAM clock gate is at 8/8
                # when the projections begin. No consumers.
                wz = pa.tile([128, 128], BF, tag="wz")
                nc.vector.memset(wz[:], 0.0)
                wu_ps = pa_ps.tile([128, 512], F32, tag="qk0", name="warmup")
                for _ in range(N_WARMUP):
                    nc.tensor.matmul(wu_ps[:, 0:128], wz[:], wz[:],
                                     start=True, stop=True)

                # Loads. Small tensors go first on the sync queue (the
                # gpsimd queue starts ~20us late due to the library load
                # and its software DGE is slow); only late-needed tensors
                # go on gpsimd.
                nc.sync.dma_start(i128_sb[:], i128d.ap())
                nc.sync.dma_start(bq_sb[:], bq2.ap())
                nc.sync.dma_start(bk_sb[:], bk2.ap())
                nc.sync.dma_start(ekt_sb[:], ekt18d.ap())
                nc.sync.dma_start(bv_sb[:], bvr.ap())
                nc.scalar.dma_start(wq_sb[:], wqt.ap().rearrange(
                    "p (s m) -> p s m", s=NSUB))
                nc.sync.dma_start(x_sb[:, 0:3, :], xb.ap().rearrange(
                    "p (s t) -> p s t", s=NSUB)[:, 0:3, :])
                nc.scalar.dma_start(x_sb[:, 3:6, :], xb.ap().rearrange(
                    "p (s t) -> p s t", s=NSUB)[:, 3:6, :])
                nc.sync.dma_start(c_sb[:, 0:3, :], cb.ap().rearrange(
                    "p (s t) -> p s t", s=NSUB)[:, 0:3, :])
                nc.scalar.dma_start(c_sb[:, 3:6, :], cb.ap().rearrange(
                    "p (s t) -> p s t", s=NSUB)[:, 3:6, :])
                nc.sync.dma_start(wk_sb[:], wkt.ap().rearrange(
                    "p (s m) -> p s m", s=NSUB))
                nc.sync.dma_start(wv_sb[:], wvt.ap().rearrange(
                    "p (s m) -> p s m", s=NSUB))
                nc.gpsimd.dma_start(wo_sb[:], wot.ap().rearrange(
                    "p (s m) -> p s m", s=LSUB))
                nc.gpsimd.dma_start(ev18_sb[:], ev18d.ap())
                for h in range(HL):
                    nc.gpsimd.dma_start(vt[h][:, :, KC:KC + 2], ones8.ap())

                # Q and K: out[dl, t] = sum_c W*T[c, dl] * x[c, t]  (+bias)
                for dst, wsb, src, bias in ((q_sb, wq_sb, x_sb, bq_sb),
                                            (k_sb, wk_sb, c_sb, bk_sb)):
                    for m in range(LSUB):
                        for n in range(2):
                            ps = pa_ps.tile([128, 512], F32,
                                            tag=f"qk{(2 * m + n) % 4}")
                            for k in range(NSUB):
                                nc.tensor.matmul(
                                    ps[:],
                                    wsb[:, k, 128 * m:128 * (m + 1)],
                                    src[:, k, 512 * n:512 * (n + 1)],
                                    start=(k == 0), stop=(k == NSUB - 1))
                            # fused copy+bias on ACT (idle in phase A)
                            nc.scalar.activation(
                                dst[:, m, 512 * n:512 * (n + 1)], ps[:],
                                AF.Identity, bias=bias[:, m:m + 1])
                    if dst is q_sb:
                        # Rt[t, j] for the head pair of subtile `sub`:
                        # stationary q-chunk [128, 128], moving block-diag
                        # ekt18 [128, 18] -> out[t, 9*hl + j]. c-outer so the
                        # band_k staging for chunk c can start early.
                        for c in range(NCH):
                            for sub in range(LSUB):
                                rt_ps = pa_ps3.tile([128, 2 * NB], F32,
                                                    tag="rt_ps")
                                nc.tensor.matmul(
                                    rt_ps[:],
                                    q_sb[:, sub, 128 * c:128 * (c + 1)],
                                    ekt_sb[:],
                                    start=True, stop=True)
                                nc.vector.tensor_copy(
                                    rts[:, c, :, 2 * sub:2 * sub + 2].transpose(
                                        [0, 2, 1]),
                                    rt_ps[:].rearrange("p (hl j) -> p hl j",
                                                       hl=2))
                            # band_k staging: shear-write Rt into per-chunk
                            # skewed regions (12B runs); chunk c-1's region is
                            # complete once chunk c's rows exist.
                            nc.scalar.dma_start(
                                _raw(rk[c].ap(), RK_GUARD - 864 * 4 + 24,
                                     [[870, 128], [864, NB], [1, HL]]),
                                rts[:, c, :, :])
                            if c > 0:
                                nc.scalar.dma_start(
                                    _raw(rk[c].ap(), 0,
                                         [[870, 4], [864, NB], [1, HL]]),
                                    rts[124:128, c - 1, :, :])
                                nc.scalar.dma_start(
                                    _raw(rk[c - 1].ap(),
                                         RK_GUARD + 864 * 124 + 792,
                                         [[870, 4], [864, NB], [1, HL]]),
                                    rts[0:4, c, :, :])
                                nc.sync.dma_start(
                                    wt6[:, c - 1, :, :],
                                    _raw(rk[c - 1].ap(), RK_GUARD,
                                         [[RK_ROW, 128], [1, 6 * WQ]]))
                        nc.sync.dma_start(
                            wt6[:, NCH - 1, :, :],
                            _raw(rk[NCH - 1].ap(), RK_GUARD,
                                 [[RK_ROW, 128], [1, 6 * WQ]]))

                # V^T: out[s, dl] = sum_c c_b[c, s] * WvT[c, dl] (+bias),
                # written per head into [128, NCH, 66] tiles, col 64 = ones.
                for c in range(NCH):
                    vt_ps = pa_ps2.tile([128, CL], F32, tag="vt_ps")
                    for k in range(NSUB):
                        nc.tensor.matmul(
                            vt_ps[:],
                            c_sb[:, k, 128 * c:128 * (c + 1)],
                            wv_sb[:, k, :],
                            start=(k == 0), stop=(k == NSUB - 1))
                    for h in range(HL):
                        nc.vector.tensor_tensor(
                            vt[h][:, c, 0:KC], vt_ps[:, KC * h:KC * (h + 1)],
                            bv_sb[:, KC * h:KC * (h + 1)], ALU.add)

            # ---------------- Phase B: attention ----------------
            segs_v = _bandv_segments()
            last_half = {}
            for idx, (c, a, b) in enumerate(segs_v):
                last_half[0 if a < 512 else 1] = idx
            with tc.tile_pool(name="pb", bufs=2) as pb, \
                 tc.tile_pool(name="pb2", bufs=2) as pb2, \
                 tc.tile_pool(name="pb3", bufs=1) as pb3, \
                 tc.tile_pool(name="pb_ps", bufs=1, space="PSUM") as pb_ps:
                for pair in range(HL // 2):
                    heads = (2 * pair, 2 * pair + 1)
                    # es[p, c, hl, t] = exp(scores^T) for the head pair;
                    # double-buffered so consecutive pairs overlap
                    es = pb.tile([128, NCH, 2, T], BF, tag="es")
                    # [128, T]: rows 0-65 = A@V (+ones denominator row 64);
                    # av0 rows 96-113 double as the pdw-transpose scratch
                    av = {hl: pb_ps.tile([128, T], F32, tag=f"av{hl}",
                                         name=f"av{heads[hl]}")
                          for hl in (0, 1)}
                    pdw6 = pb2.tile([128, NCH, 2 * NB], BF, tag="pdw6")
                    # shear target; zeros outside the written diagonals
                    pdc = pb2.tile([2 * NB, NCH, WQ], BF, tag="pdc")
                    nc.vector.memset(pdc[:], 0.0)

                    # per-chunk pdw transpose goes into the spare partitions
                    # (96-113) of the av0 PSUM banks; copied to SBUF pmsb
                    # per chunk so the end-of-pair tail is short.
                    pmsb = pb3.tile([2 * NB, T], BF, tag="pmsb")

                    def emit_av(c):
                        # A@V for a full chunk: one vt stationary load per
                        # head serves both 512-column halves.
                        for hl in (0, 1):
                            for n in (0, 1):
                                nc.tensor.matmul(
                                    av[hl][0:KC + 2, 512 * n:512 * (n + 1)],
                                    vt[heads[hl]][:, c, :],
                                    es[:, c, hl, 512 * n:512 * (n + 1)],
                                    start=(c == 0), stop=False,
                                    skip_group_check=True)

                    def emit_pmt(c):
                        nc.tensor.matmul(
                            av[0][96:96 + 2 * NB, 128 * c:128 * (c + 1)],
                            pdw6[:, c, :],
                            i128_sb[:],
                            start=True, stop=True, skip_group_check=True,
                            tile_position=(0, 96))
                        nc.vector.tensor_copy(
                            pmsb[:, 128 * c:128 * (c + 1)],
                            av[0][96:96 + 2 * NB, 128 * c:128 * (c + 1)])

                    for c in range(NCH):
                        t_lo, q0, w = _chunk_window(c)
                        t0 = 128 * c - WIN
                        stn = {n: pb_ps.tile([128, T], F32, tag=f"st{n}",
                                             name=f"st{pair}_{c}_{n}")
                               for n in (0, 1)}
                        segs = {n: _half_segments(c, n) for n in (0, 1)}
                        # scores: one k-chunk stationary load per head
                        # serves both halves; the two heads' matmuls use
                        # disjoint PE row groups (rb 0/64).
                        for hl in (0, 1):
                            rb = 64 * hl
                            for n in (0, 1):
                                nc.tensor.matmul(
                                    stn[n][:, 512 * hl:512 * (hl + 1)],
                                    k_sb[rb:rb + 64, pair,
                                         128 * c:128 * (c + 1)],
                                    q_sb[rb:rb + 64, pair,
                                         512 * n:512 * (n + 1)],
                                    start=True, stop=(not segs[n]))
                        # band_k: accumulate the window into the scores
                        # PSUM group as an identity-stationary matmul
                        for n in (0, 1):
                            for a, b in segs[n]:
                                for hl in (0, 1):
                                    nc.tensor.matmul(
                                        stn[n][:, 512 * hl + a - 512 * n:
                                               512 * hl + b - 512 * n],
                                        i128_sb[:],
                                        wt6[:, c, a - t0:b - t0, heads[hl]],
                                        start=False, stop=True,
                                        skip_group_check=True)
                            # softmax numerator, both heads in one op
                            nc.scalar.activation(
                                es[:, c, :, 512 * n:512 * (n + 1)],
                                stn[n][:].rearrange("p (hl t) -> p hl t",
                                                    hl=2),
                                AF.Exp)
                        # A@V one chunk late so the in-order PE queue never
                        # blocks on exp(c); scores(c+1) fills the gap.
                        if c > 0:
                            emit_av(c - 1)
                        # pdw transpose three chunks late (covers the es
                        # window staging round-trip latency)
                        if c >= 3:
                            emit_pmt(c - 3)
                        # stage both heads' es windows (272B runs) and read
                        # back the compact diagonals (18B runs):
                        # pdw6[p, c, 9*hl+i] = es_hl[p, t0 + p + i]
                        buf = w2[pair * NCH + c].ap()
                        nc.sync.dma_start(
                            _raw(buf, 8 * W2_COLS + q0,
                                 [[W2_COLS, 128], [W2_REG, 2], [1, w]]),
                            es[:, c, :, t_lo:t_lo + w])
                        nc.sync.dma_start(
                            pdw6[:, c, :],
                            _raw(buf, 8 * W2_COLS,
                                 [[W2_COLS + 1, 128], [W2_REG, 2],
                                  [1, NB]]))
                    emit_av(NCH - 1)
                    for c in range(NCH - 3, NCH):
                        emit_pmt(c)

                    # start the reciprocal chain as soon as the last A@V
                    # lands: the denominator row (64) is untouched by the
                    # band matmuls, so only the final multiply must wait.
                    rlrs = {}
                    for hl in (0, 1):
                        ll = pb3.tile([1, T], F32, tag=f"ll{hl}")
                        nc.vector.tensor_copy(ll[:], av[hl][KC:KC + 1, :])
                        lr8 = pb3.tile([128, 8], F32, tag=f"lr8{hl}")
                        nc.sync.dma_start(
                            lr8[:],
                            ll[:].rearrange("o (p k) -> o p k", p=128))
                        lr8r = pb3.tile([128, 8], F32, tag=f"lr8r{hl}")
                        nc.vector.reciprocal(lr8r[:], lr8[:])
                        rl = pb3.tile([1, T], F32, tag=f"rl{hl}")
                        nc.sync.dma_start(
                            rl[:].rearrange("o (p k) -> o p k", p=128),
                            lr8r[:])
                        rlr = pb3.tile([KC, T], F32, tag=f"rlr{hl}")
                        nc.gpsimd.partition_broadcast(rlr[:], rl[:])
                        rlrs[hl] = rlr

                    # shear-align with row permute folded into the src AP:
                    # pdc[2i+hl, c, i+p] = pm[9hl+i, 128c+p]
                    pmsb_r = pmsb[:].rearrange("(hl i) t -> i hl t", i=NB)
                    for i in range(NB):
                        eng = (nc.sync, nc.gpsimd)[i % 2]
                        eng.dma_start(
                            pdc[2 * i:2 * i + 2, :, i:i + 128],
                            pmsb_r[i].rearrange("hl (c p) -> hl c p", c=NCH))
                    # band_v: av[d, t] += sum_i ev[8-i, d] * pdc[2i+hl, t]
                    for hl in (0, 1):
                        for idx, (c, a, b) in enumerate(segs_v):
                            t0 = 128 * c - WIN
                            nc.tensor.matmul(
                                av[hl][0:KC, a:b],
                                ev18_sb[:, KC * hl:KC * (hl + 1)],
                                pdc[:, c, a - t0:b - t0],
                                start=False,
                                stop=(idx == last_half[0 if a < 512 else 1]),
                                skip_group_check=True)

                    # normalize by the denominator row and merge heads
                    for hl in (0, 1):
                        rows = 64 * hl
                        nc.vector.tensor_tensor(
                            merged[rows:rows + KC, pair, :],
                            av[hl][0:KC, :], rlrs[hl][:], ALU.mult)

                # ------------ Phase C: output projection ------------
                # (same PSUM pool: o_ps reuses the st banks so the first
                # m-tiles overlap the last pair's band/normalize tail)
                with tc.tile_pool(name="pc", bufs=3) as pc:
                    for m in range(NSUB):
                        ps = pb_ps.tile([128, T], F32, tag=f"st{m % 2}",
                                        name=f"o_ps{m}")
                        for n in range(2):
                            for k in range(LSUB):
                                nc.tensor.matmul(
                                    ps[:, 512 * n:512 * (n + 1)],
                                    wo_sb[:, k, 128 * m:128 * (m + 1)],
                                    merged[:, k, 512 * n:512 * (n + 1)],
                                    start=(k == 0), stop=(k == LSUB - 1))
                        for n in range(2):
                            ot = pc.tile([128, 512], F16, tag="o_sb")
                            if (2 * m + n) % 2 == 0:
                                nc.vector.tensor_copy(
                                    ot[:], ps[:, 512 * n:512 * (n + 1)])
                            else:
                                nc.scalar.activation(
                                    ot[:], ps[:, 512 * n:512 * (n + 1)],
                                    AF.Identity)
                            eng = (nc.sync, nc.gpsimd,
                                   nc.scalar)[(2 * m + n) % 3]
                            eng.dma_start(
                                outp.ap()[128 * m:128 * (m + 1),
                                          512 * n:512 * (n + 1)],
                                ot[:])

    nc.compile()
    return nc


_CACHE = {}


def _get_program():
    if "nc" not in _CACHE:
        _CACHE["nc"] = build_program()
    return _CACHE["nc"]


def _tile128(a):
    """[(s*128), m] -> [128, s*m] partition-major host pre-tiling, bf16."""
    s = a.shape[0] // 128
    return np.ascontiguousarray(
        a.reshape(s, 128, a.shape[1]).transpose(1, 0, 2).reshape(
            128, s * a.shape[1])).astype(NPBF)


def _prep_core_inputs(core, x, c, Wq, bq, Wk, bk, Wv, bv, Wo,
                      emb_rel_k, emb_rel_v, zeros_rk, zeros_w2):
    b, hg = core // 2, core % 2
    hsl = slice(hg * CL, (hg + 1) * CL)
    scale = KC ** -0.5
    ek = np.ascontiguousarray(emb_rel_k[0])  # [9, 64]
    ekt = np.ascontiguousarray(ek.T)         # [64, 9]
    ev = np.ascontiguousarray(emb_rel_v[0])  # [9, 64]
    ekt18 = np.zeros((128, 2 * NB), np.float32)
    ekt18[0:KC, 0:NB] = ekt
    ekt18[KC:128, NB:2 * NB] = ekt
    ev18 = np.zeros((2 * NB, 128), np.float32)
    for i in range(NB):
        for hl in range(2):
            ev18[2 * i + hl, KC * hl:KC * (hl + 1)] = ev[NB - 1 - i]
    ins = {
        "ones8": np.concatenate([np.ones((128, NCH, 1), np.float32),
                                 np.zeros((128, NCH, 1), np.float32)],
                                axis=2).astype(NPBF),
        "xb": _tile128(np.asarray(x[b])),
        "cb": _tile128(np.asarray(c[b])),
        "wqt": _tile128(np.ascontiguousarray((Wq[hsl] * scale).T)),
        "wkt": _tile128(np.ascontiguousarray(Wk[hsl].T)),
        "wvt": _tile128(np.ascontiguousarray(Wv[hsl].T)),
        "wot": _tile128(np.ascontiguousarray(Wo[:, hsl].T)),
        "bq2": np.ascontiguousarray((bq[hsl] * scale).reshape(LSUB, 128).T),
        "bk2": np.ascontiguousarray(bk[hsl].reshape(LSUB, 128).T),
        "bvr": np.ascontiguousarray(np.tile(bv[hsl][None, :], (128, 1))),
        "ekt18": ekt18.astype(NPBF),
        "ev18": ev18.astype(NPBF),
        "i128": np.eye(128, dtype=np.float32).astype(NPBF),
    }
    for ch in range(NCH):
        ins[f"rk{ch}"] = zeros_rk
    for i in range((HL // 2) * NCH):
        ins[f"w2_{i}"] = zeros_w2
    return ins


def kernel(**inputs):
    inputs = {k: np.asarray(v, dtype=np.float32) for k, v in inputs.items()}
    nc = _get_program()
    zeros_rk = np.zeros(RK_LEN, NPBF)
    zeros_w2 = np.zeros(2 * W2_REG, NPBF)
    in_maps = [
        _prep_core_inputs(
            core, inputs["x"], inputs["c"],
            inputs["Wq"], inputs["bq"], inputs["Wk"], inputs["bk"],
            inputs["Wv"], inputs["bv"], inputs["Wo"],
            inputs["emb_rel_k"], inputs["emb_rel_v"],
            zeros_rk, zeros_w2)
        for core in range(8)
    ]
    res = run_bass_kernel_spmd(nc, in_maps, core_ids=list(range(8)),
                               **_CACHE.get("run_kwargs", {}))
    _CACHE["last_result"] = res
    parts = [np.asarray(r["outp"], dtype=np.float32) for r in res.results]
    bo = inputs["bo"]
    out = np.stack([parts[2 * b] + parts[2 * b + 1] + bo[:, None]
                    for b in range(B)])
    return out.astype(np.float32)


# revision 24
# speedup vs baseline: 1.0337x; 1.0337x over previous
"""Trainium2 Bass kernel for nn_MultiHeadAttention_42752104464925.

Multi-head attention (VITS-style) with windowed relative position embeddings
(window=4, heads_share=True).

Math notes
----------
With L=1024, WIN=4, the relative-key logits term rel_to_abs(q_scaled @ rel_k^T)
is a 9-diagonal band:   scores[t,s] += q_scaled[t] . emb_k[s-t+4]   (|s-t|<=4)
and the relative-value term is:
  out[t] += sum_j p[t, t+j-4] * emb_v[j]   (0 <= t+j-4 < L)

Sharding: 8 cores = 4 batches x 2 head-groups (6 heads each). Each core
computes QKV for its 384 channels, attention for its 6 heads, and a partial
output projection Wo[:, slice] @ merged. Host sums the two partials per batch.

Device layout per head: scores are computed TRANSPOSED (S^T[s,t], s on
partitions) so that A@V needs no transpose of the attention weights; the
softmax denominator L[t] (a partition-dim sum) comes for free from a ones
column appended to the V^T stationary operand (M=66).

Schedule notes (perf):
* The whole datapath is bf16 (fp32 PSUM accumulation, fp32 output): halves
  the HBM load traffic (which paces phase A) and the band staging DMAs, and
  frees enough SBUF to double-buffer the exp(scores) tile so consecutive
  head pairs overlap.
* All large DRAM inputs are host-pre-tiled to [128, ...] partition-major so
  each load is 128 long descriptor runs; small tensors load on the sync
  queue first (the gpsimd queue starts ~20us late due to the library load).
* Identity warmup matmuls run during the input DMAs so the PE HAM clock
  gate is at 8/8 when the projections start.
* band_k is applied on the PE: the per-chunk band window wt6 (staged via the
  skewed DRAM regions rk*) is accumulated into the scores PSUM group as an
  identity-stationary matmul, so the scores->exp chain never leaves
  TensorE->ScalarE.
* A@V matmuls for half-chunk (c,n) are emitted one half-chunk late so the
  in-order PE queue never blocks on exp(c,n); scores(c,n+1) fills the gap.
* band_v: the 9 diagonals of exp(S) are read back from the es window staging
  as a compact [128, 18] tile, transposed on TensorE, shear-aligned with 9
  SBUF->SBUF DMAs (row-permuting with stride-9 partition pairs), and folded
  into the A@V PSUM accumulation with K=18 matmuls against a
  host-precomputed embedding matrix.
"""

import math

import numpy as np
import ml_dtypes

import concourse.bacc as bacc
import concourse.bass as bass
import concourse.mybir as mybir
import concourse.tile as tile
from concourse import library_config
from concourse.bass_utils import run_bass_kernel_spmd

# Problem constants (hardcoded per harness contract).
B, C, T, H, KC, WIN = 4, 768, 1024, 12, 64, 4
HL = 6            # heads per core
CL = HL * KC      # 384 local channels
NSUB = C // 128   # 6 k-subtiles over C
LSUB = CL // 128  # 3 subtiles over CL
NCH = T // 128    # 8 s-chunks
NB = 9            # band width (2*WIN+1)
WQ = 136          # band window width per 128-chunk (128 + 2*WIN)
# band_k skewed staging: region element addr = RK_GUARD + RK_ROW*p + 6*f + h
# holds W[p, f, h] = Rt_h[t0+f, p-f+8]; rows are 816 elements used of RK_ROW.
RK_ROW = 6 * 144
RK_GUARD = 6912
RK_LEN = RK_GUARD + RK_ROW * WQ
W2_ROWS, W2_COLS = 144, 136  # padded expS window staging (8 zero rows each end)
W2_REG = W2_ROWS * W2_COLS   # one head's region; a pair shares one tensor

F32 = mybir.dt.float32
BF = mybir.dt.bfloat16
NPBF = ml_dtypes.bfloat16
AF = mybir.ActivationFunctionType
ALU = mybir.AluOpType

N_WARMUP = 200     # warmup matmuls issued during the load phase
F16 = mybir.dt.float16


def _raw(t_ap, off, dims):
    """Raw element-offset AP into (the tensor behind) an AP."""
    return bass.AP(tensor=t_ap.tensor, offset=t_ap.offset + off, ap=dims)


def _chunk_window(c):
    """Clipped t-window [t_lo, t_lo+w) for s-chunk c; q0 = offset into the
    unclipped 136-wide window starting at t0 = 128c - 4."""
    t0 = 128 * c - WIN
    t_lo = max(0, t0)
    q0 = t_lo - t0
    w = min(T, t0 + WQ) - t_lo
    return t_lo, q0, w


def _half_segments(c, n):
    """Absolute-t segments of chunk c's band window inside half n."""
    t_lo, q0, w = _chunk_window(c)
    a = max(t_lo, 512 * n)
    b = min(t_lo + w, 512 * (n + 1))
    return [(a, b)] if a < b else []


def _bandv_segments():
    """(c, a, b) absolute-t segments of each chunk's band window, split at
    PSUM bank (512) boundaries."""
    segs = []
    for c in range(NCH):
        t0 = 128 * c - WIN
        a, b = max(t0, 0), min(t0 + WQ, T)
        cuts = [a] + [x for x in (512,) if a < x < b] + [b]
        for k in range(len(cuts) - 1):
            segs.append((c, cuts[k], cuts[k + 1]))
    return segs


def build_program():
    nc = bacc.Bacc("TRN2", target_bir_lowering=False, debug=False,
                   enable_asserts=True)

    # ---- I/O ----  (big tensors host-pre-tiled to [128, ...], bf16)
    xb = nc.dram_tensor("xb", [128, NSUB * T], BF, kind="ExternalInput")
    cb = nc.dram_tensor("cb", [128, NSUB * T], BF, kind="ExternalInput")
    wqt = nc.dram_tensor("wqt", [128, NSUB * CL], BF, kind="ExternalInput")
    wkt = nc.dram_tensor("wkt", [128, NSUB * CL], BF, kind="ExternalInput")
    wvt = nc.dram_tensor("wvt", [128, NSUB * CL], BF, kind="ExternalInput")
    wot = nc.dram_tensor("wot", [128, LSUB * C], BF, kind="ExternalInput")
    bq2 = nc.dram_tensor("bq2", [128, LSUB], F32, kind="ExternalInput")
    bk2 = nc.dram_tensor("bk2", [128, LSUB], F32, kind="ExternalInput")
    bvr = nc.dram_tensor("bvr", [128, CL], F32, kind="ExternalInput")
    ekt18d = nc.dram_tensor("ekt18", [128, 2 * NB], BF, kind="ExternalInput")
    ev18d = nc.dram_tensor("ev18", [2 * NB, 128], BF, kind="ExternalInput")
    i128d = nc.dram_tensor("i128", [128, 128], BF, kind="ExternalInput")
    ones8 = nc.dram_tensor("ones8", [128, NCH, 2], BF, kind="ExternalInput")
    # zero-padded staging buffers (host supplies zeros; device writes data)
    rk = [nc.dram_tensor(f"rk{c}", [RK_LEN], BF, kind="ExternalInput")
          for c in range(NCH)]
    w2 = [nc.dram_tensor(f"w2_{i}", [2 * W2_REG], BF, kind="ExternalInput")
          for i in range((HL // 2) * NCH)]
    outp = nc.dram_tensor("outp", [C, T], F16, kind="ExternalOutput")

    with tile.TileContext(nc) as tc:
        nc.gpsimd.load_library(library_config.attn)
        with tc.tile_pool(name="persist", bufs=1) as pp:
            # persistent SBUF
            q_sb = pp.tile([128, LSUB, T], BF, tag="q_sb")
            k_sb = pp.tile([128, LSUB, T], BF, tag="k_sb")
            vt = [pp.tile([128, NCH, KC + 2], BF, tag=f"vt{h}", name=f"vt{h}")
                  for h in range(HL)]
            wo_sb = pp.tile([128, LSUB, C], BF, tag="wo_sb")
            merged = pp.tile([128, LSUB, T], BF, tag="merged")
            ekt_sb = pp.tile([128, 2 * NB], BF, tag="ekt_sb")
            ev18_sb = pp.tile([2 * NB, 128], BF, tag="ev18_sb")
            i128_sb = pp.tile([128, 128], BF, tag="i128_sb")
            bq_sb = pp.tile([128, LSUB], F32, tag="bq_sb")
            bk_sb = pp.tile([128, LSUB], F32, tag="bk_sb")
            bv_sb = pp.tile([128, CL], F32, tag="bv_sb")
            # band_k windows for all chunks/heads: [p, c, f, h]
            wt6 = pp.tile([128, NCH, WQ, HL], BF, tag="wt6")

            # ---------------- Phase A: projections ----------------
            with tc.tile_pool(name="pa", bufs=1) as pa, \
                 tc.tile_pool(name="pa_ps", bufs=1, space="PSUM") as pa_ps, \
                 tc.tile_pool(name="pa_ps2", bufs=2, space="PSUM") as pa_ps2, \
                 tc.tile_pool(name="pa_ps3", bufs=2, space="PSUM") as pa_ps3:
                x_sb = pa.tile([128, NSUB, T], BF, tag="x_sb")
                c_sb = pa.tile([128, NSUB, T], BF, tag="c_sb")
                wq_sb = pa.tile([128, NSUB, CL], BF, tag="wq_sb")
                wk_sb = pa.tile([128, NSUB, CL], BF, tag="wk_sb")
                wv_sb = pa.tile([128, NSUB, CL], BF, tag="wv_sb")
                # Rt staging, head-interleaved: [t_part, c, j, h]
                rts = pa.tile([128, NCH, NB, HL], BF, tag="rts")

                # PE warmup: a memset SBUF tile needs no DMA, so the PE
                # starts within ~1.5us and the HAM clock gate is at 8/8
                # when the projections begin. No consumers.
                wz = pa.tile([128, 128], BF, tag="wz")
                nc.vector.memset(wz[:], 0.0)
                wu_ps = pa_ps.tile([128, 512], F32, tag="qk0", name="warmup")
                for _ in range(N_WARMUP):
                    nc.tensor.matmul(wu_ps[:, 0:128], wz[:], wz[:],
                                     start=True, stop=True)

                # Loads. Small tensors go first on the sync queue (the
                # gpsimd queue starts ~20us late due to the library load
                # and its software DGE is slow); only late-needed tensors
                # go on gpsimd.
                nc.sync.dma_start(i128_sb[:], i128d.ap())
                nc.sync.dma_start(bq_sb[:], bq2.ap())
                nc.sync.dma_start(bk_sb[:], bk2.ap())
                nc.sync.dma_start(ekt_sb[:], ekt18d.ap())
                nc.scalar.dma_start(wq_sb[:], wqt.ap().rearrange(
                    "p (s m) -> p s m", s=NSUB))
                nc.sync.dma_start(x_sb[:, 0:3, :], xb.ap().rearrange(
                    "p (s t) -> p s t", s=NSUB)[:, 0:3, :])
                nc.scalar.dma_start(x_sb[:, 3:6, :], xb.ap().rearrange(
                    "p (s t) -> p s t", s=NSUB)[:, 3:6, :])
                nc.sync.dma_start(c_sb[:, 0:3, :], cb.ap().rearrange(
                    "p (s t) -> p s t", s=NSUB)[:, 0:3, :])
                nc.scalar.dma_start(c_sb[:, 3:6, :], cb.ap().rearrange(
                    "p (s t) -> p s t", s=NSUB)[:, 3:6, :])
                nc.sync.dma_start(wk_sb[:], wkt.ap().rearrange(
                    "p (s m) -> p s m", s=NSUB))
                nc.scalar.dma_start(wv_sb[:], wvt.ap().rearrange(
                    "p (s m) -> p s m", s=NSUB))
                nc.gpsimd.dma_start(bv_sb[:], bvr.ap())
                nc.gpsimd.dma_start(ev18_sb[:], ev18d.ap())
                for h in range(HL):
                    nc.gpsimd.dma_start(vt[h][:, :, KC:KC + 2], ones8.ap())
                nc.gpsimd.dma_start(wo_sb[:], wot.ap().rearrange(
                    "p (s m) -> p s m", s=LSUB))

                # Q and K: out[dl, t] = sum_c W*T[c, dl] * x[c, t]  (+bias)
                for dst, wsb, src, bias in ((q_sb, wq_sb, x_sb, bq_sb),
                                            (k_sb, wk_sb, c_sb, bk_sb)):
                    for m in range(LSUB):
                        for n in range(2):
                            ps = pa_ps.tile([128, 512], F32,
                                            tag=f"qk{(2 * m + n) % 4}")
                            for k in range(NSUB):
                                nc.tensor.matmul(
                                    ps[:],
                                    wsb[:, k, 128 * m:128 * (m + 1)],
                                    src[:, k, 512 * n:512 * (n + 1)],
                                    start=(k == 0), stop=(k == NSUB - 1))
                            # fused copy+bias on ACT (idle in phase A)
                            nc.scalar.activation(
                                dst[:, m, 512 * n:512 * (n + 1)], ps[:],
                                AF.Identity, bias=bias[:, m:m + 1])
                    if dst is q_sb:
                        # Rt[t, j] for the head pair of subtile `sub`:
                        # stationary q-chunk [128, 128], moving block-diag
                        # ekt18 [128, 18] -> out[t, 9*hl + j]. c-outer so the
                        # band_k staging for chunk c can start early.
                        for c in range(NCH):
                            for sub in range(LSUB):
                                rt_ps = pa_ps3.tile([128, 2 * NB], F32,
                                                    tag="rt_ps")
                                nc.tensor.matmul(
                                    rt_ps[:],
                                    q_sb[:, sub, 128 * c:128 * (c + 1)],
                                    ekt_sb[:],
                                    start=True, stop=True)
                                nc.vector.tensor_copy(
                                    rts[:, c, :, 2 * sub:2 * sub + 2].transpose(
                                        [0, 2, 1]),
                                    rt_ps[:].rearrange("p (hl j) -> p hl j",
                                                       hl=2))
                            # band_k staging: shear-write Rt into per-chunk
                            # skewed regions (12B runs); chunk c-1's region is
                            # complete once chunk c's rows exist.
                            nc.scalar.dma_start(
                                _raw(rk[c].ap(), RK_GUARD - 864 * 4 + 24,
                                     [[870, 128], [864, NB], [1, HL]]),
                                rts[:, c, :, :])
                            if c > 0:
                                nc.scalar.dma_start(
                                    _raw(rk[c].ap(), 0,
                                         [[870, 4], [864, NB], [1, HL]]),
                                    rts[124:128, c - 1, :, :])
                                nc.scalar.dma_start(
                                    _raw(rk[c - 1].ap(),
                                         RK_GUARD + 864 * 124 + 792,
                                         [[870, 4], [864, NB], [1, HL]]),
                                    rts[0:4, c, :, :])
                                nc.sync.dma_start(
                                    wt6[:, c - 1, :, :],
                                    _raw(rk[c - 1].ap(), RK_GUARD,
                                         [[RK_ROW, 128], [1, 6 * WQ]]))
                        nc.sync.dma_start(
                            wt6[:, NCH - 1, :, :],
                            _raw(rk[NCH - 1].ap(), RK_GUARD,
                                 [[RK_ROW, 128], [1, 6 * WQ]]))

                # V^T: out[s, dl] = sum_c c_b[c, s] * WvT[c, dl] (+bias),
                # written per head into [128, NCH, 66] tiles, col 64 = ones.
                for c in range(NCH):
                    vt_ps = pa_ps2.tile([128, CL], F32, tag="vt_ps")
                    for k in range(NSUB):
                        nc.tensor.matmul(
                            vt_ps[:],
                            c_sb[:, k, 128 * c:128 * (c + 1)],
                            wv_sb[:, k, :],
                            start=(k == 0), stop=(k == NSUB - 1))
                    for h in range(HL):
                        nc.vector.tensor_tensor(
                            vt[h][:, c, 0:KC], vt_ps[:, KC * h:KC * (h + 1)],
                            bv_sb[:, KC * h:KC * (h + 1)], ALU.add)

            # ---------------- Phase B: attention ----------------
            segs_v = _bandv_segments()
            last_half = {}
            for idx, (c, a, b) in enumerate(segs_v):
                last_half[0 if a < 512 else 1] = idx
            with tc.tile_pool(name="pb", bufs=2) as pb, \
                 tc.tile_pool(name="pb2", bufs=2) as pb2, \
                 tc.tile_pool(name="pb3", bufs=1) as pb3, \
                 tc.tile_pool(name="pb_ps", bufs=1, space="PSUM") as pb_ps:
                for pair in range(HL // 2):
                    heads = (2 * pair, 2 * pair + 1)
                    # es[p, c, hl, t] = exp(scores^T) for the head pair;
                    # double-buffered so consecutive pairs overlap
                    es = pb.tile([128, NCH, 2, T], BF, tag="es")
                    # [128, T]: rows 0-65 = A@V (+ones denominator row 64);
                    # av0 rows 96-113 double as the pdw-transpose scratch
                    av = {hl: pb_ps.tile([128, T], F32, tag=f"av{hl}",
                                         name=f"av{heads[hl]}")
                          for hl in (0, 1)}
                    pdw6 = pb2.tile([128, NCH, 2 * NB], BF, tag="pdw6")
                    # shear target; zeros outside the written diagonals
                    pdc = pb2.tile([2 * NB, NCH, WQ], BF, tag="pdc")
                    nc.vector.memset(pdc[:], 0.0)

                    # per-chunk pdw transpose goes into the spare partitions
                    # (96-113) of the av0 PSUM banks; copied to SBUF pmsb
                    # per chunk so the end-of-pair tail is short.
                    pmsb = pb3.tile([2 * NB, T], BF, tag="pmsb")

                    def emit_av(c):
                        # A@V for a full chunk: one vt stationary load per
                        # head serves both 512-column halves.
                        for hl in (0, 1):
                            for n in (0, 1):
                                nc.tensor.matmul(
                                    av[hl][0:KC + 2, 512 * n:512 * (n + 1)],
                                    vt[heads[hl]][:, c, :],
                                    es[:, c, hl, 512 * n:512 * (n + 1)],
                                    start=(c == 0), stop=False,
                                    skip_group_check=True)

                    def emit_pmt(c):
                        nc.tensor.matmul(
                            av[0][96:96 + 2 * NB, 128 * c:128 * (c + 1)],
                            pdw6[:, c, :],
                            i128_sb[:],
                            start=True, stop=True, skip_group_check=True,
                            tile_position=(0, 96))
                        nc.vector.tensor_copy(
                            pmsb[:, 128 * c:128 * (c + 1)],
                            av[0][96:96 + 2 * NB, 128 * c:128 * (c + 1)])

                    for c in range(NCH):
                        t_lo, q0, w = _chunk_window(c)
                        t0 = 128 * c - WIN
                        stn = {n: pb_ps.tile([128, T], F32, tag=f"st{n}",
                                             name=f"st{pair}_{c}_{n}")
                               for n in (0, 1)}
                        segs = {n: _half_segments(c, n) for n in (0, 1)}
                        # scores: one k-chunk stationary load per head
                        # serves both halves; the two heads' matmuls use
                        # disjoint PE row groups (rb 0/64).
                        for hl in (0, 1):
                            rb = 64 * hl
                            for n in (0, 1):
                                nc.tensor.matmul(
                                    stn[n][:, 512 * hl:512 * (hl + 1)],
                                    k_sb[rb:rb + 64, pair,
                                         128 * c:128 * (c + 1)],
                                    q_sb[rb:rb + 64, pair,
                                         512 * n:512 * (n + 1)],
                                    start=True, stop=(not segs[n]))
                        # band_k: accumulate the window into the scores
                        # PSUM group as an identity-stationary matmul
                        for n in (0, 1):
                            for a, b in segs[n]:
                                for hl in (0, 1):
                                    nc.tensor.matmul(
                                        stn[n][:, 512 * hl + a - 512 * n:
                                               512 * hl + b - 512 * n],
                                        i128_sb[:],
                                        wt6[:, c, a - t0:b - t0, heads[hl]],
                                        start=False, stop=True,
                                        skip_group_check=True)
                            # softmax numerator, both heads in one op
                            nc.scalar.activation(
                                es[:, c, :, 512 * n:512 * (n + 1)],
                                stn[n][:].rearrange("p (hl t) -> p hl t",
                                                    hl=2),
                                AF.Exp)
                        # A@V one chunk late so the in-order PE queue never
                        # blocks on exp(c); scores(c+1) fills the gap.
                        if c > 0:
                            emit_av(c - 1)
                        # pdw transpose three chunks late (covers the es
                        # window staging round-trip latency)
                        if c >= 3:
                            emit_pmt(c - 3)
                        # stage both heads' es windows (272B runs) and read
                        # back the compact diagonals (18B runs):
                        # pdw6[p, c, 9*hl+i] = es_hl[p, t0 + p + i]
                        buf = w2[pair * NCH + c].ap()
                        nc.gpsimd.dma_start(
                            _raw(buf, 8 * W2_COLS + q0,
                                 [[W2_COLS, 128], [W2_REG, 2], [1, w]]),
                            es[:, c, :, t_lo:t_lo + w])
                        nc.sync.dma_start(
                            pdw6[:, c, :],
                            _raw(buf, 8 * W2_COLS,
                                 [[W2_COLS + 1, 128], [W2_REG, 2],
                                  [1, NB]]))
                    emit_av(NCH - 1)
                    for c in range(NCH - 3, NCH):
                        emit_pmt(c)

                    # start the reciprocal chain as soon as the last A@V
                    # lands: the denominator row (64) is untouched by the
                    # band matmuls, so only the final multiply must wait.
                    rlrs = {}
                    for hl in (0, 1):
                        ll = pb3.tile([1, T], F32, tag=f"ll{hl}")
                        nc.vector.tensor_copy(ll[:], av[hl][KC:KC + 1, :])
                        lr8 = pb3.tile([128, 8], F32, tag=f"lr8{hl}")
                        nc.sync.dma_start(
                            lr8[:],
                            ll[:].rearrange("o (p k) -> o p k", p=128))
                        lr8r = pb3.tile([128, 8], F32, tag=f"lr8r{hl}")
                        nc.vector.reciprocal(lr8r[:], lr8[:])
                        rl = pb3.tile([1, T], F32, tag=f"rl{hl}")
                        nc.sync.dma_start(
                            rl[:].rearrange("o (p k) -> o p k", p=128),
                            lr8r[:])
                        rlr = pb3.tile([KC, T], F32, tag=f"rlr{hl}")
                        nc.gpsimd.partition_broadcast(rlr[:], rl[:])
                        rlrs[hl] = rlr

                    # shear-align with row permute folded into the src AP:
                    # pdc[2i+hl, c, i+p] = pm[9hl+i, 128c+p]
                    pmsb_r = pmsb[:].rearrange("(hl i) t -> i hl t", i=NB)
                    for i in range(NB):
                        eng = (nc.sync, nc.gpsimd)[i % 2]
                        eng.dma_start(
                            pdc[2 * i:2 * i + 2, :, i:i + 128],
                            pmsb_r[i].rearrange("hl (c p) -> hl c p", c=NCH))
                    # band_v: av[d, t] += sum_i ev[8-i, d] * pdc[2i+hl, t]
                    for hl in (0, 1):
                        for idx, (c, a, b) in enumerate(segs_v):
                            t0 = 128 * c - WIN
                            nc.tensor.matmul(
                                av[hl][0:KC, a:b],
                                ev18_sb[:, KC * hl:KC * (hl + 1)],
                                pdc[:, c, a - t0:b - t0],
                                start=False,
                                stop=(idx == last_half[0 if a < 512 else 1]),
                                skip_group_check=True)

                    # normalize by the denominator row and merge heads
                    for hl in (0, 1):
                        rows = 64 * hl
                        nc.vector.tensor_tensor(
                            merged[rows:rows + KC, pair, :],
                            av[hl][0:KC, :], rlrs[hl][:], ALU.mult)

                # ------------ Phase C: output projection ------------
                # (same PSUM pool: o_ps reuses the st banks so the first
                # m-tiles overlap the last pair's band/normalize tail)
                with tc.tile_pool(name="pc", bufs=3) as pc:
                    for m in range(NSUB):
                        ps = pb_ps.tile([128, T], F32, tag=f"st{m % 2}",
                                        name=f"o_ps{m}")
                        for n in range(2):
                            for k in range(LSUB):
                                nc.tensor.matmul(
                                    ps[:, 512 * n:512 * (n + 1)],
                                    wo_sb[:, k, 128 * m:128 * (m + 1)],
                                    merged[:, k, 512 * n:512 * (n + 1)],
                                    start=(k == 0), stop=(k == LSUB - 1))
                        for n in range(2):
                            ot = pc.tile([128, 512], F16, tag="o_sb")
                            if (2 * m + n) % 2 == 0:
                                nc.vector.tensor_copy(
                                    ot[:], ps[:, 512 * n:512 * (n + 1)])
                            else:
                                nc.scalar.activation(
                                    ot[:], ps[:, 512 * n:512 * (n + 1)],
                                    AF.Identity)
                            eng = (nc.sync, nc.gpsimd,
                                   nc.scalar)[(2 * m + n) % 3]
                            eng.dma_start(
                                outp.ap()[128 * m:128 * (m + 1),
                                          512 * n:512 * (n + 1)],
                                ot[:])

    nc.compile()
    return nc


_CACHE = {}


def _get_program():
    if "nc" not in _CACHE:
        _CACHE["nc"] = build_program()
    return _CACHE["nc"]


def _tile128(a):
    """[(s*128), m] -> [128, s*m] partition-major host pre-tiling, bf16."""
    s = a.shape[0] // 128
    return np.ascontiguousarray(
        a.reshape(s, 128, a.shape[1]).transpose(1, 0, 2).reshape(
            128, s * a.shape[1])).astype(NPBF)


def _prep_core_inputs(core, x, c, Wq, bq, Wk, bk, Wv, bv, Wo,
                      emb_rel_k, emb_rel_v, zeros_rk, zeros_w2):
    b, hg = core // 2, core % 2
    hsl = slice(hg * CL, (hg + 1) * CL)
    scale = KC ** -0.5
    ek = np.ascontiguousarray(emb_rel_k[0])  # [9, 64]
    ekt = np.ascontiguousarray(ek.T)         # [64, 9]
    ev = np.ascontiguousarray(emb_rel_v[0])  # [9, 64]
    ekt18 = np.zeros((128, 2 * NB), np.float32)
    ekt18[0:KC, 0:NB] = ekt
    ekt18[KC:128, NB:2 * NB] = ekt
    ev18 = np.zeros((2 * NB, 128), np.float32)
    for i in range(NB):
        for hl in range(2):
            ev18[2 * i + hl, KC * hl:KC * (hl + 1)] = ev[NB - 1 - i]
    ins = {
        "ones8": np.concatenate([np.ones((128, NCH, 1), np.float32),
                                 np.zeros((128, NCH, 1), np.float32)],
                                axis=2).astype(NPBF),
        "xb": _tile128(np.asarray(x[b])),
        "cb": _tile128(np.asarray(c[b])),
        "wqt": _tile128(np.ascontiguousarray((Wq[hsl] * scale).T)),
        "wkt": _tile128(np.ascontiguousarray(Wk[hsl].T)),
        "wvt": _tile128(np.ascontiguousarray(Wv[hsl].T)),
        "wot": _tile128(np.ascontiguousarray(Wo[:, hsl].T)),
        "bq2": np.ascontiguousarray((bq[hsl] * scale).reshape(LSUB, 128).T),
        "bk2": np.ascontiguousarray(bk[hsl].reshape(LSUB, 128).T),
        "bvr": np.ascontiguousarray(np.tile(bv[hsl][None, :], (128, 1))),
        "ekt18": ekt18.astype(NPBF),
        "ev18": ev18.astype(NPBF),
        "i128": np.eye(128, dtype=np.float32).astype(NPBF),
    }
    for ch in range(NCH):
        ins[f"rk{ch}"] = zeros_rk
    for i in range((HL // 2) * NCH):
        ins[f"w2_{i}"] = zeros_w2
    return ins


def kernel(**inputs):
    inputs = {k: np.asarray(v, dtype=np.float32) for k, v in inputs.items()}
    nc = _get_program()
    zeros_rk = np.zeros(RK_LEN, NPBF)
    zeros_w2 = np.zeros(2 * W2_REG, NPBF)
    in_maps = [
        _prep_core_inputs(
            core, inputs["x"], inputs["c"],
            inputs["Wq"], inputs["bq"], inputs["Wk"], inputs["bk"],
            inputs["Wv"], inputs["bv"], inputs["Wo"],
            inputs["emb_rel_k"], inputs["emb_rel_v"],
            zeros_rk, zeros_w2)
        for core in range(8)
    ]
    res = run_bass_kernel_spmd(nc, in_maps, core_ids=list(range(8)),
                               **_CACHE.get("run_kwargs", {}))
    _CACHE["last_result"] = res
    parts = [np.asarray(r["outp"], dtype=np.float32) for r in res.results]
    bo = inputs["bo"]
    out = np.stack([parts[2 * b] + parts[2 * b + 1] + bo[:, None]
                    for b in range(B)])
    return out.astype(np.float32)
